# revision 28
# baseline (speedup 1.0000x reference)
"""Trainium2 Bass kernel for 2-layer GraphSAGE (mean aggregation), v3.

8-core SPMD, 64-node windows (1648 global, 206/core), layer-1 pull with
host-pregathered messages, layer-2 push over local sources with a
ReduceScatter:

- L1: host slots each window's in-edges into K1 128-edge tiles and
  pre-gathers x[src] bf16; device builds [128,64] one-hot M tiles on DVE
  (is_equal x weight), PSUM-accumulates feature-major agg per 4-window
  super, W1 matmuls + bias + ReLU -> h1T [64, 13184] bf16.
- h1 rows: HWDGE dma-transposes per window pair -> h1rows [128, 103*64];
  one strided DMA writes a 256B-padded local row table [13184, 128] bf16.
- L2: edges grouped by SOURCE core; per global dst window, gather local
  h1 rows by int16 row index (batched dma_gather, 256B elems), build
  [128,64] one-hot M2 (dst slot x 1/cnt) and matmul with M2 as lhsT ->
  ROW-major partials [64r, 64f]; two windows pack one PSUM [128,*] tile;
  staged bf16 partials DMA to a [105472, 64] table; one ReduceScatter
  leaves each core its own rows' agg2.
- Final: load agg2 rows, PE-transpose per pair to feature-major, W2
  matmuls + bias -> out [64, 13184] fp32; host un-permutes.
"""
import sys

sys.path.insert(0, '/opt/trn_rl_repo')
import heapq

import numpy as np
import ml_dtypes

BF16 = ml_dtypes.bfloat16
N = 100000
D = 64
NCORES = 8
P = 128
WIN = 64                    # nodes per window
WPC = 206                   # windows per core
PAIRS = WPC // 2            # 103 window pairs per core
WROWS = WPC * WIN           # 13184 local rows
NWIN = NCORES * WPC         # 1648 global windows
TBL_ROWS = NCORES * WROWS   # 105472 rows in the partial table

CHUNK1_W = 12               # L1 windows per streamed msgs chunk
SUPER1_W = 4                # L1 windows per PSUM super (<= 4; 4*64=256 cols)
L2_TILE_CAP = 112           # max edge-tiles per L2 gather chunk
L2_GROUP_PAIRS = 4          # window pairs per L2 PSUM tile ([128, 4*64])
RS_SPLIT = 48               # local pairs in the first ReduceScatter chunk


def _balance_nodes(deg):
    """Assign nodes to NWIN windows of <=WIN nodes, balancing degree sums."""
    order = np.argsort(-deg, kind='stable')
    win = np.empty(N, dtype=np.int32)
    slot = np.empty(N, dtype=np.int32)
    counts = np.zeros(NWIN, dtype=np.int32)
    heap = [(0, w) for w in range(NWIN)]
    heapq.heapify(heap)
    for n in order:
        while True:
            load, w = heapq.heappop(heap)
            if counts[w] < WIN:
                break
        win[n] = w
        slot[n] = counts[w]
        counts[w] += 1
        load += int(deg[n])
        if counts[w] < WIN:
            heapq.heappush(heap, (load, w))
    return win, slot


def _wrap_idx(flat):
    """int16 gather index layout: per 128-idx tile, idx j -> partition j%16
    (replicated x8 across 128 partitions), column j//16."""
    nt = flat.shape[0] // P
    v = flat.reshape(nt, 8, 16).transpose(2, 0, 1).reshape(16, nt * 8)
    return np.tile(v, (8, 1)).astype(np.int16)


def _host_prep(x, edge_index):
    x = np.asarray(x, dtype=np.float32)
    edge_index = np.asarray(edge_index)
    src = edge_index[0].astype(np.int64)
    dst = edge_index[1].astype(np.int64)
    cnt = np.bincount(dst, minlength=N).astype(np.float32)
    inv = (1.0 / np.maximum(cnt, 1.0)).astype(np.float32)
    win, slot = _balance_nodes(cnt.astype(np.int64))
    srow = (win % WPC) * WIN + slot            # local table row of each node
    score = win // WPC                          # owning core of each node

    # ---- L1 (pull): per-core edge tiles grouped by dst window ----
    dwin = win[dst]
    wcnt = np.bincount(dwin, minlength=NWIN)
    K1 = max(1, int(np.max((wcnt + P - 1) // P)))
    T1 = WPC * K1

    order1 = np.lexsort((srow[src], dwin))
    dwin_s = dwin[order1]
    dslot_s = slot[dst][order1].astype(np.float32)
    w_s = inv[dst][order1]
    esrc_s = src[order1]
    starts1 = np.searchsorted(dwin_s, np.arange(NWIN + 1))

    # ---- L2 (push): per-core edges grouped by global dst window ----
    ecore = score[src]                          # core owning the edge source
    key2 = ecore.astype(np.int64) * NWIN + dwin
    order2 = np.lexsort((srow[src], key2))
    k2_s = key2[order2]
    q_s = srow[src][order2].astype(np.int16)    # local src row (gather idx)
    dslot2_s = slot[dst][order2].astype(np.float32)
    w2_s = inv[dst][order2]
    starts2 = np.searchsorted(k2_s, np.arange(NCORES * NWIN + 1))
    cnt2 = (starts2[1:] - starts2[:-1]).reshape(NCORES, NWIN)
    K2 = np.maximum(1, (cnt2.max(axis=0) + P - 1) // P).astype(np.int32)  # [NWIN]
    tile0 = np.zeros(NWIN + 1, dtype=np.int64)
    tile0[1:] = np.cumsum(K2)
    T2 = int(tile0[-1])

    per_core = []
    for c in range(NCORES):
        # L1 slots
        s_dstloc = np.full(T1 * P, -1.0, dtype=np.float32)
        s_w = np.zeros(T1 * P, dtype=np.float32)
        s_esrc = np.zeros(T1 * P, dtype=np.int64)
        for wl in range(WPC):
            g = c * WPC + wl
            s0, s1 = starts1[g], starts1[g + 1]
            n = s1 - s0
            base = wl * K1 * P
            s_dstloc[base:base + n] = dslot_s[s0:s1]
            s_w[base:base + n] = w_s[s0:s1]
            s_esrc[base:base + n] = esrc_s[s0:s1]

        def to_pt(a, dt, T):
            return np.ascontiguousarray(a.reshape(T, P).T.astype(dt))

        msgs = x[s_esrc].astype(BF16)
        msgs_pt = np.ascontiguousarray(
            msgs.reshape(T1, P, D).transpose(1, 0, 2).reshape(P, T1 * D))

        # L2 slots
        q2 = np.zeros(T2 * P, dtype=np.int16)
        s_dstloc2 = np.full(T2 * P, -1.0, dtype=np.float32)
        s_w2 = np.zeros(T2 * P, dtype=np.float32)
        for g in range(NWIN):
            i = c * NWIN + g
            s0, s1 = starts2[i], starts2[i + 1]
            n = s1 - s0
            base = int(tile0[g]) * P
            q2[base:base + n] = q_s[s0:s1]
            s_dstloc2[base:base + n] = dslot2_s[s0:s1]
            s_w2[base:base + n] = w2_s[s0:s1]

        # local_scatter M1-build inputs: idx = k*WIN + dstloc (pad -> -1)
        dst_pt = to_pt(s_dstloc, np.float32, T1)
        w_pt = to_pt(s_w, np.float32, T1)
        k_of_t = np.tile(np.arange(K1, dtype=np.float32), WPC)
        ls_idx = np.where(dst_pt >= 0, dst_pt + k_of_t[None, :] * WIN,
                          -1.0).astype(np.int16)

        # xT: col wl*64 + s = x[node at (win c*WPC+wl, slot s)]
        per_core.append({
            "msgs": msgs_pt,
            "dstloc": dst_pt,
            "wts": w_pt,
            "lsidx": ls_idx,
            "lsw": w_pt.astype(BF16),
            "qidx": _wrap_idx(q2),
            "dstloc2": to_pt(s_dstloc2, np.float32, T2),
            "wts2": to_pt(s_w2, np.float32, T2),
        })

    colmap = np.full((NCORES, WROWS), -1, dtype=np.int64)
    colmap[score, srow] = np.arange(N)
    for c in range(NCORES):
        cm = colmap[c]
        xT = np.zeros((D, WROWS), dtype=BF16)
        used = cm >= 0
        xT[:, used] = x[cm[used]].T.astype(BF16)
        per_core[c]["xT"] = xT
    return per_core, K1, K2, win, slot


def _l2_chunks(K2):
    """Split the 824 global window pairs into gather chunks of whole
    PSUM groups (L2_GROUP_PAIRS pairs) with <= L2_TILE_CAP tiles.

    Two phases: first every core-range's local pairs [0, RS_SPLIT) (feeds
    the early ReduceScatter), then the rest."""
    pair_tiles = K2.reshape(NWIN // 2, 2).sum(axis=1)
    ranges = [(PAIRS * c, PAIRS * c + RS_SPLIT) for c in range(NCORES)]
    ranges += [(PAIRS * c + RS_SPLIT, PAIRS * (c + 1)) for c in range(NCORES)]
    chunks = []  # (pair0, npair, ntiles)
    for r0, r1 in ranges:
        p0 = r0
        while p0 < r1:
            p1 = p0
            tiles = 0
            while p1 < r1 and p1 - p0 + L2_GROUP_PAIRS <= 56:
                g1 = min(p1 + L2_GROUP_PAIRS, r1)
                add = int(pair_tiles[p1:g1].sum())
                if tiles + add > L2_TILE_CAP and tiles > 0:
                    break
                tiles += add
                p1 = g1
            chunks.append((p0, p1 - p0, tiles))
            p0 = p1
    return chunks


def _build_program(K1, K2, chunks):
    import concourse.bass as bass
    import concourse.tile as tile
    from concourse import bacc, mybir
    from concourse import library_config

    T1 = WPC * K1
    tile0 = np.zeros(NWIN + 1, dtype=np.int64)
    tile0[1:] = np.cumsum(K2)
    T2 = int(tile0[-1])

    nc = bacc.Bacc("TRN2", target_bir_lowering=False, debug=False,
                   num_devices=NCORES)
    dt = mybir.dt

    msgs_d = nc.dram_tensor("msgs", [P, T1 * D], dt.bfloat16, kind="ExternalInput")
    dstloc_d = nc.dram_tensor("dstloc", [P, T1], dt.float32, kind="ExternalInput")
    wts_d = nc.dram_tensor("wts", [P, T1], dt.float32, kind="ExternalInput")
    lsidx_d = nc.dram_tensor("lsidx", [P, T1], dt.int16, kind="ExternalInput")
    lsw_d = nc.dram_tensor("lsw", [P, T1], dt.bfloat16, kind="ExternalInput")
    qidx_d = nc.dram_tensor("qidx", [P, T2 * 8], dt.int16, kind="ExternalInput")
    dstloc2_d = nc.dram_tensor("dstloc2", [P, T2], dt.float32, kind="ExternalInput")
    wts2_d = nc.dram_tensor("wts2", [P, T2], dt.float32, kind="ExternalInput")
    xT_d = nc.dram_tensor("xT", [D, WROWS], dt.bfloat16, kind="ExternalInput")
    iota_d = nc.dram_tensor("iota", [P, WIN], dt.bfloat16, kind="ExternalInput")
    ident_d = nc.dram_tensor("ident", [P, P], dt.bfloat16, kind="ExternalInput")
    w1l_d = nc.dram_tensor("w1lT", [D, D], dt.bfloat16, kind="ExternalInput")
    w1r_d = nc.dram_tensor("w1rT", [D, D], dt.bfloat16, kind="ExternalInput")
    w2l_d = nc.dram_tensor("w2lT", [D, D], dt.bfloat16, kind="ExternalInput")
    w2r_d = nc.dram_tensor("w2rT", [D, D], dt.bfloat16, kind="ExternalInput")
    b1_d = nc.dram_tensor("b1c", [D, 1], dt.float32, kind="ExternalInput")
    b2_d = nc.dram_tensor("b2c", [D, 1], dt.float32, kind="ExternalInput")
    out_d = nc.dram_tensor("out", [D, WROWS], dt.float32, kind="ExternalOutput")

    # L1 chunks of CHUNK1_W windows
    l1_chunks = []
    w0 = 0
    while w0 < WPC:
        cw = min(CHUNK1_W, WPC - w0)
        l1_chunks.append((w0, cw))
        w0 += cw

    with tile.TileContext(nc) as tc:
        with (
            tc.tile_pool(name="const", bufs=1) as cpool,
            tc.tile_pool(name="dram", bufs=1, space="DRAM") as dpool,
        ):
            iota_sb = cpool.tile([P, WIN], dt.bfloat16, tag="iota")
            ident_sb = cpool.tile([P, P], dt.bfloat16, tag="ident")
            w1l_sb = cpool.tile([D, D], dt.bfloat16, tag="w1l")
            w1r_sb = cpool.tile([D, D], dt.bfloat16, tag="w1r")
            w2l_sb = cpool.tile([D, D], dt.bfloat16, tag="w2l")
            w2r_sb = cpool.tile([D, D], dt.bfloat16, tag="w2r")
            b1_sb = cpool.tile([D, 1], dt.float32, tag="b1")
            b2_sb = cpool.tile([D, 1], dt.float32, tag="b2")
            h1T_sb = cpool.tile([D, WROWS], dt.bfloat16, tag="h1T")
            h1rows_sb = cpool.tile([P, PAIRS * D], dt.bfloat16, tag="h1rows")
            qidx_sb = cpool.tile([P, T2 * 8], dt.int16, tag="qidx")
            dstloc2_sb = cpool.tile([P, T2], dt.float32, tag="dstloc2")
            wts2_sb = cpool.tile([P, T2], dt.float32, tag="wts2")

            table_dram = dpool.tile([WROWS, P], dt.bfloat16, tag="table")
            # two partial tables, one per ReduceScatter chunk; rows are
            # core-major so the flat 8-way RS split lands on core boundaries
            NPA, NPB = RS_SPLIT, PAIRS - RS_SPLIT
            partialA = dpool.tile([NCORES * NPA * P, D], dt.bfloat16, tag="pA")
            partialB = dpool.tile([NCORES * NPB * P, D], dt.bfloat16, tag="pB")
            agg2A = dpool.tile([NPA * P, D], dt.bfloat16, tag="agg2A")
            agg2B = dpool.tile([NPB * P, D], dt.bfloat16, tag="agg2B")

            # ---------------- layer 1 ----------------
            with (
                tc.tile_pool(name="l1c", bufs=1) as l1c,
                tc.tile_pool(name="ch", bufs=2) as chpool,
                tc.tile_pool(name="m1", bufs=16) as mpool,
                tc.tile_pool(name="sp1", bufs=3) as spool,
                tc.tile_pool(name="psA", bufs=2, space="PSUM") as psA,
                tc.tile_pool(name="psB", bufs=2, space="PSUM") as psB,
            ):
                dstloc_sb = l1c.tile([P, T1], dt.float32, tag="dstloc")
                wts_sb = l1c.tile([P, T1], dt.float32, tag="wts")
                lsidx_sb = l1c.tile([P, T1], dt.int16, tag="lsidx")
                lsw_sb = l1c.tile([P, T1], dt.bfloat16, tag="lsw")
                xT_sb = l1c.tile([D, WROWS], dt.bfloat16, tag="xT")

                for t_sb, t_d in [(iota_sb, iota_d), (dstloc_sb, dstloc_d),
                                  (wts_sb, wts_d)]:
                    nc.sync.dma_start(out=t_sb[:], in_=t_d.ap())
                for t_sb, t_d in [(lsidx_sb, lsidx_d), (lsw_sb, lsw_d)]:
                    nc.scalar.dma_start(out=t_sb[:], in_=t_d.ap())
                # L2 consts load on idle engines during L1 (CoreSim charges
                # DMA transfers serially to the issuing engine)
                deferred_sp = [(ident_sb, ident_d), (w1l_sb, w1l_d),
                               (w1r_sb, w1r_d), (b1_sb, b1_d),
                               (w2l_sb, w2l_d), (w2r_sb, w2r_d), (b2_sb, b2_d)]
                deferred_pool = [(qidx_sb, qidx_d), (dstloc2_sb, dstloc2_d),
                                 (wts2_sb, wts2_d)]
                deferred_act = [(xT_sb, xT_d)]

                for wci, (w0, cw) in enumerate(l1_chunks):
                    ch = chpool.tile([P, CHUNK1_W * K1 * D], dt.bfloat16, tag="ch")
                    (nc.sync if wci % 2 == 0 else nc.scalar).dma_start(
                        out=ch[:, :cw * K1 * D],
                        in_=msgs_d.ap()[:, w0 * K1 * D:(w0 + cw) * K1 * D])
                    if w0 == 0:
                        for t_sb, t_d in deferred_sp:
                            nc.sync.dma_start(out=t_sb[:], in_=t_d.ap())
                        for t_sb, t_d in deferred_pool:
                            nc.gpsimd.dma_start(out=t_sb[:], in_=t_d.ap())
                        for t_sb, t_d in deferred_act:
                            nc.scalar.dma_start(out=t_sb[:], in_=t_d.ap())
                    s0 = 0
                    while s0 < cw:
                        sw = min(SUPER1_W, cw - s0)
                        agg_ps = psA.tile([D, SUPER1_W * WIN], dt.float32, tag="agg")
                        for s in range(sw):
                            wi = w0 + s0 + s
                            if wi % 2 == 0:
                                # whole-window M build on the idle Pool engine
                                mwin = mpool.tile([P, K1 * WIN], dt.bfloat16,
                                                  tag="Mw")
                                nc.gpsimd.local_scatter(
                                    out_ap=mwin[:],
                                    data_ap=lsw_sb[:, wi * K1:(wi + 1) * K1],
                                    idxs_ap=lsidx_sb[:, wi * K1:(wi + 1) * K1],
                                    channels=P, num_elems=K1 * WIN,
                                    num_idxs=K1)
                            for k in range(K1):
                                t = wi * K1 + k
                                if wi % 2 == 0:
                                    mt = mwin[:, k * WIN:(k + 1) * WIN]
                                else:
                                    m1t = mpool.tile([P, WIN], dt.bfloat16,
                                                     tag="M")
                                    nc.vector.tensor_scalar(
                                        out=m1t[:], in0=iota_sb[:],
                                        scalar1=dstloc_sb[:, t:t + 1],
                                        scalar2=wts_sb[:, t:t + 1],
                                        op0=mybir.AluOpType.is_equal,
                                        op1=mybir.AluOpType.mult)
                                    mt = m1t[:]
                                woff = s0 + s
                                nc.tensor.matmul(
                                    out=agg_ps[:, s * WIN:(s + 1) * WIN],
                                    lhsT=ch[:, (woff * K1 + k) * D:
                                            (woff * K1 + k + 1) * D],
                                    rhs=mt, start=(k == 0), stop=(k == K1 - 1))
                        agg_sb = spool.tile([D, SUPER1_W * WIN], dt.bfloat16,
                                            tag="aggsb")
                        nc.scalar.copy(out=agg_sb[:, :sw * WIN],
                                       in_=agg_ps[:, :sw * WIN])
                        h_ps = psB.tile([D, SUPER1_W * WIN], dt.float32, tag="hps")
                        wabs = w0 + s0
                        nc.tensor.matmul(out=h_ps[:, :sw * WIN], lhsT=w1l_sb[:],
                                         rhs=agg_sb[:, :sw * WIN],
                                         start=True, stop=False)
                        nc.tensor.matmul(out=h_ps[:, :sw * WIN], lhsT=w1r_sb[:],
                                         rhs=xT_sb[:, wabs * WIN:(wabs + sw) * WIN],
                                         start=False, stop=True)
                        nc.scalar.activation(
                            out=h1T_sb[:, wabs * WIN:(wabs + sw) * WIN],
                            in_=h_ps[:, :sw * WIN],
                            func=mybir.ActivationFunctionType.Relu, bias=b1_sb[:])
                        s0 += sw

                # rows for the table: pair j -> local rows j*128 + p
                for j in range(PAIRS):
                    nc.sync.dma_start_transpose(
                        out=h1rows_sb[:, j * D:(j + 1) * D],
                        in_=h1T_sb[:, j * P:(j + 1) * P])
                tbl_v = table_dram[:].rearrange("(j p) e -> p j e", p=P)
                rows_v = h1rows_sb[:].rearrange("p (j f) -> p j f", f=D)
                HALF = PAIRS // 2
                nc.sync.dma_start(out=tbl_v[:, :HALF, 0:D],
                                  in_=rows_v[:, :HALF, :])
                nc.scalar.dma_start(out=tbl_v[:, HALF:, 0:D],
                                    in_=rows_v[:, HALF:, :])

            # ---------------- layer 2 (push + ReduceScatter) ----------------
            nc.gpsimd.load_library(library_config.mlp)
            stg_engines = [nc.sync, nc.scalar]
            last_p1 = max(i for i, (p0, _, _) in enumerate(chunks)
                          if p0 % PAIRS < RS_SPLIT)
            with (
                tc.tile_pool(name="gq", bufs=2) as gqpool,
                tc.tile_pool(name="m2", bufs=16) as m2pool,
                tc.tile_pool(name="stg", bufs=2) as stgpool,
                tc.tile_pool(name="psP", bufs=3, space="PSUM") as psP,
            ):
                for ci, (p0, npair, ntiles) in enumerate(chunks):
                    t0 = int(tile0[2 * p0])
                    gq = gqpool.tile([P, L2_TILE_CAP * P], dt.bfloat16, tag="gq")
                    nc.gpsimd.dma_gather(
                        gq[:, :ntiles * P].rearrange("p (c e) -> p c e", e=P),
                        table_dram[:], qidx_sb[:, t0 * 8:(t0 + ntiles) * 8],
                        ntiles * P, ntiles * P, P, single_packet=False)
                    stg = stgpool.tile([P, 56 * D], dt.bfloat16, tag="stg")
                    g0 = 0
                    while g0 < npair:
                        gp = min(L2_GROUP_PAIRS, npair - g0)
                        pps = psP.tile([P, L2_GROUP_PAIRS * D], dt.float32,
                                       tag="pps")
                        for pr in range(gp):
                            pair = p0 + g0 + pr
                            for h in range(2):
                                g = 2 * pair + h
                                kk = int(K2[g])
                                tg = int(tile0[g])
                                for k in range(kk):
                                    t = tg + k
                                    mt = m2pool.tile([P, WIN], dt.bfloat16,
                                                     tag="M2")
                                    nc.vector.tensor_scalar(
                                        out=mt[:], in0=iota_sb[:],
                                        scalar1=dstloc2_sb[:, t:t + 1],
                                        scalar2=wts2_sb[:, t:t + 1],
                                        op0=mybir.AluOpType.is_equal,
                                        op1=mybir.AluOpType.mult)
                                    nc.tensor.matmul(
                                        out=pps[h * WIN:(h + 1) * WIN,
                                                pr * D:(pr + 1) * D],
                                        lhsT=mt[:],
                                        rhs=gq[:, (t - t0) * P:(t - t0) * P + D],
                                        start=(k == 0), stop=(k == kk - 1))
                        nc.scalar.copy(out=stg[:, g0 * D:(g0 + gp) * D],
                                       in_=pps[:, :gp * D])
                        g0 += gp
                    c, jl = p0 // PAIRS, p0 % PAIRS
                    if jl < RS_SPLIT:
                        tgt, row0 = partialA, c * NPA + jl
                    else:
                        tgt, row0 = partialB, c * NPB + (jl - RS_SPLIT)
                    stg_engines[ci % 2].dma_start(
                        out=tgt[:].rearrange("(j p) f -> p j f", p=P)
                            [:, row0:row0 + npair, :],
                        in_=stg[:, :npair * D].rearrange("p (j f) -> p j f", f=D))
                    if ci == last_p1:
                        # RS1 slots between phase-1 and phase-2 gathers on Pool
                        nc.gpsimd.collective_compute(
                            "ReduceScatter", mybir.AluOpType.add,
                            replica_groups=[list(range(NCORES))],
                            ins=[partialA[:]], outs=[agg2A[:]])

            # -------- final: W2r*h1 during the collectives, then W2l --------
            SPLIT = RS_SPLIT * P
            with (
                tc.tile_pool(name="fin", bufs=1) as fin,
                tc.tile_pool(name="psT", bufs=2, space="PSUM") as psT,
                tc.tile_pool(name="psC", bufs=2, space="PSUM") as psC,
            ):
                a2rows = fin.tile([P, PAIRS * D], dt.bfloat16, tag="a2rows")
                a2T = fin.tile([D, WROWS], dt.bfloat16, tag="a2T")
                h2r = fin.tile([D, WROWS], dt.bfloat16, tag="h2r")
                ot = fin.tile([D, WROWS], dt.float32, tag="ot")

                # h2r = W2r @ h1 + b2 — independent of the collectives
                for s0 in range(0, WROWS, 512):
                    sw = min(512, WROWS - s0)
                    h_ps = psC.tile([D, 512], dt.float32, tag="h2rps")
                    nc.tensor.matmul(out=h_ps[:, :sw], lhsT=w2r_sb[:],
                                     rhs=h1T_sb[:, s0:s0 + sw],
                                     start=True, stop=True)
                    nc.scalar.add(out=h2r[:, s0:s0 + sw], in_=h_ps[:, :sw],
                                  add=b2_sb[:])

                nc.gpsimd.collective_compute(
                    "ReduceScatter", mybir.AluOpType.add,
                    replica_groups=[list(range(NCORES))],
                    ins=[partialB[:]], outs=[agg2B[:]])

                halves = [(0, RS_SPLIT, agg2A), (RS_SPLIT, PAIRS, agg2B)]
                for j0, j1, a2d in halves:
                    nc.sync.dma_start(
                        out=a2rows[:, j0 * D:j1 * D].rearrange(
                            "p (j f) -> p j f", f=D),
                        in_=a2d[:].rearrange("(j p) f -> p j f", p=P))
                    for jg in range(j0, j1, 4):
                        je = min(jg + 4, j1)
                        tr = psT.tile([D, 4 * P], dt.bfloat16, tag="tr")
                        for j in range(jg, je):
                            nc.tensor.transpose(
                                out=tr[:, (j - jg) * P:(j - jg + 1) * P],
                                in_=a2rows[:, j * D:(j + 1) * D],
                                identity=ident_sb[:])
                        nc.scalar.copy(out=a2T[:, jg * P:je * P],
                                       in_=tr[:, :(je - jg) * P])
                    for s0 in range(j0 * P, j1 * P, 512):
                        sw = min(512, j1 * P - s0)
                        h_ps = psC.tile([D, 512], dt.float32, tag="h2ps")
                        nc.tensor.matmul(out=h_ps[:, :sw], lhsT=w2l_sb[:],
                                         rhs=a2T[:, s0:s0 + sw],
                                         start=True, stop=True)
                        nc.vector.scalar_tensor_tensor(
                            out=ot[:, s0:s0 + sw], in0=h_ps[:, :sw],
                            scalar=1.0, in1=h2r[:, s0:s0 + sw],
                            op0=mybir.AluOpType.mult,
                            op1=mybir.AluOpType.add)
                    nc.sync.dma_start(out=out_d.ap()[:, j0 * P:j1 * P],
                                      in_=ot[:, j0 * P:j1 * P])

    nc.compile()
    return nc


def prepare(x, edge_index, W1l, W1r, b1, W2l, W2r, b2):
    per_core, K1, K2, win, slot = _host_prep(x, edge_index)
    iota = np.tile(np.arange(WIN, dtype=np.float32), (P, 1)).astype(BF16)
    ident = np.eye(P, dtype=np.float32).astype(BF16)
    common = {
        "iota": iota, "ident": ident,
        "w1lT": np.asarray(W1l, np.float32).T.astype(BF16).copy(),
        "w1rT": np.asarray(W1r, np.float32).T.astype(BF16).copy(),
        "w2lT": np.asarray(W2l, np.float32).T.astype(BF16).copy(),
        "w2rT": np.asarray(W2r, np.float32).T.astype(BF16).copy(),
        "b1c": np.asarray(b1, np.float32).reshape(D, 1).copy(),
        "b2c": np.asarray(b2, np.float32).reshape(D, 1).copy(),
    }
    in_maps = [{**common, **pc} for pc in per_core]
    chunks = _l2_chunks(K2)
    nc = _build_program(K1, K2, chunks)
    return nc, in_maps, win, slot


def kernel(x, edge_index, W1l, W1r, b1, W2l, W2r, b2):
    from concourse import bass_utils

    nc, in_maps, win, slot = prepare(x, edge_index, W1l, W1r, b1,
                                     W2l, W2r, b2)
    res = bass_utils.run_bass_kernel_spmd(nc, in_maps, list(range(NCORES)))

    out = np.empty((N, D), dtype=np.float32)
    cols = (win % WPC) * WIN + slot
    cores = win // WPC
    for c in range(NCORES):
        m = cores == c
        out[m] = res.results[c]["out"][:, cols[m]].T
    return out


# revision 29
# speedup vs baseline: 1.0634x; 1.0634x over previous
"""Trainium2 Bass kernel for 2-layer GraphSAGE (mean aggregation), v3.

8-core SPMD, 64-node windows (1648 global, 206/core), layer-1 pull with
host-pregathered messages, layer-2 push over local sources with a
ReduceScatter:

- L1: host slots each window's in-edges into K1 128-edge tiles and
  pre-gathers x[src] bf16; device builds [128,64] one-hot M tiles on DVE
  (is_equal x weight), PSUM-accumulates feature-major agg per 4-window
  super, W1 matmuls + bias + ReLU -> h1T [64, 13184] bf16.
- h1 rows: HWDGE dma-transposes per window pair -> h1rows [128, 103*64];
  one strided DMA writes a 256B-padded local row table [13184, 128] bf16.
- L2: edges grouped by SOURCE core; per global dst window, gather local
  h1 rows by int16 row index (batched dma_gather, 256B elems), build
  [128,64] one-hot M2 (dst slot x 1/cnt) and matmul with M2 as lhsT ->
  ROW-major partials [64r, 64f]; two windows pack one PSUM [128,*] tile;
  staged bf16 partials DMA to a [105472, 64] table; one ReduceScatter
  leaves each core its own rows' agg2.
- Final: load agg2 rows, PE-transpose per pair to feature-major, W2
  matmuls + bias -> out [64, 13184] fp32; host un-permutes.
"""
import sys

sys.path.insert(0, '/opt/trn_rl_repo')
import heapq

import numpy as np
import ml_dtypes

BF16 = ml_dtypes.bfloat16
N = 100000
D = 64
NCORES = 8
P = 128
WIN = 64                    # nodes per window
WPC = 206                   # windows per core
PAIRS = WPC // 2            # 103 window pairs per core
WROWS = WPC * WIN           # 13184 local rows
NWIN = NCORES * WPC         # 1648 global windows
TBL_ROWS = NCORES * WROWS   # 105472 rows in the partial table

CHUNK1_W = 12               # L1 windows per streamed msgs chunk
SUPER1_W = 4                # L1 windows per PSUM super (<= 4; 4*64=256 cols)
L2_TILE_CAP = 112           # max edge-tiles per L2 gather chunk
L2_GROUP_PAIRS = 4          # window pairs per L2 PSUM tile ([128, 4*64])
RS_SPLIT = 48               # local pairs in the first ReduceScatter chunk


def _balance_nodes(deg):
    """Assign nodes to NWIN windows of <=WIN nodes, balancing degree sums."""
    order = np.argsort(-deg, kind='stable')
    win = np.empty(N, dtype=np.int32)
    slot = np.empty(N, dtype=np.int32)
    counts = np.zeros(NWIN, dtype=np.int32)
    heap = [(0, w) for w in range(NWIN)]
    heapq.heapify(heap)
    for n in order:
        while True:
            load, w = heapq.heappop(heap)
            if counts[w] < WIN:
                break
        win[n] = w
        slot[n] = counts[w]
        counts[w] += 1
        load += int(deg[n])
        if counts[w] < WIN:
            heapq.heappush(heap, (load, w))
    return win, slot


def _wrap_idx(flat):
    """int16 gather index layout: per 128-idx tile, idx j -> partition j%16
    (replicated x8 across 128 partitions), column j//16."""
    nt = flat.shape[0] // P
    v = flat.reshape(nt, 8, 16).transpose(2, 0, 1).reshape(16, nt * 8)
    return np.tile(v, (8, 1)).astype(np.int16)


def _host_prep(x, edge_index):
    x = np.asarray(x, dtype=np.float32)
    edge_index = np.asarray(edge_index)
    src = edge_index[0].astype(np.int64)
    dst = edge_index[1].astype(np.int64)
    cnt = np.bincount(dst, minlength=N).astype(np.float32)
    inv = (1.0 / np.maximum(cnt, 1.0)).astype(np.float32)
    win, slot = _balance_nodes(cnt.astype(np.int64))
    srow = (win % WPC) * WIN + slot            # local table row of each node
    score = win // WPC                          # owning core of each node

    # ---- L1 (pull): per-core edge tiles grouped by dst window ----
    dwin = win[dst]
    wcnt = np.bincount(dwin, minlength=NWIN)
    K1 = max(1, int(np.max((wcnt + P - 1) // P)))
    T1 = WPC * K1

    order1 = np.lexsort((srow[src], dwin))
    dwin_s = dwin[order1]
    dslot_s = slot[dst][order1].astype(np.float32)
    w_s = inv[dst][order1]
    esrc_s = src[order1]
    starts1 = np.searchsorted(dwin_s, np.arange(NWIN + 1))

    # ---- L2 (push): per-core edges grouped by global dst window ----
    ecore = score[src]                          # core owning the edge source
    key2 = ecore.astype(np.int64) * NWIN + dwin
    order2 = np.lexsort((srow[src], key2))
    k2_s = key2[order2]
    q_s = srow[src][order2].astype(np.int16)    # local src row (gather idx)
    dslot2_s = slot[dst][order2].astype(np.float32)
    w2_s = inv[dst][order2]
    starts2 = np.searchsorted(k2_s, np.arange(NCORES * NWIN + 1))
    cnt2 = (starts2[1:] - starts2[:-1]).reshape(NCORES, NWIN)
    K2 = np.maximum(1, (cnt2.max(axis=0) + P - 1) // P).astype(np.int32)  # [NWIN]
    tile0 = np.zeros(NWIN + 1, dtype=np.int64)
    tile0[1:] = np.cumsum(K2)
    T2 = int(tile0[-1])

    per_core = []
    for c in range(NCORES):
        # L1 slots
        s_dstloc = np.full(T1 * P, -1.0, dtype=np.float32)
        s_w = np.zeros(T1 * P, dtype=np.float32)
        s_esrc = np.zeros(T1 * P, dtype=np.int64)
        for wl in range(WPC):
            g = c * WPC + wl
            s0, s1 = starts1[g], starts1[g + 1]
            n = s1 - s0
            base = wl * K1 * P
            s_dstloc[base:base + n] = dslot_s[s0:s1]
            s_w[base:base + n] = w_s[s0:s1]
            s_esrc[base:base + n] = esrc_s[s0:s1]

        def to_pt(a, dt, T):
            return np.ascontiguousarray(a.reshape(T, P).T.astype(dt))

        msgs = x[s_esrc].astype(BF16)
        msgs_pt = np.ascontiguousarray(
            msgs.reshape(T1, P, D).transpose(1, 0, 2).reshape(P, T1 * D))

        # L2 slots
        q2 = np.zeros(T2 * P, dtype=np.int16)
        s_dstloc2 = np.full(T2 * P, -1.0, dtype=np.float32)
        s_w2 = np.zeros(T2 * P, dtype=np.float32)
        for g in range(NWIN):
            i = c * NWIN + g
            s0, s1 = starts2[i], starts2[i + 1]
            n = s1 - s0
            base = int(tile0[g]) * P
            q2[base:base + n] = q_s[s0:s1]
            s_dstloc2[base:base + n] = dslot2_s[s0:s1]
            s_w2[base:base + n] = w2_s[s0:s1]

        # local_scatter M1-build inputs: idx = k*WIN + dstloc (pad -> -1)
        dst_pt = to_pt(s_dstloc, np.float32, T1)
        w_pt = to_pt(s_w, np.float32, T1)
        k_of_t = np.tile(np.arange(K1, dtype=np.float32), WPC)
        ls_idx = np.where(dst_pt >= 0, dst_pt + k_of_t[None, :] * WIN,
                          -1.0).astype(np.int16)

        # xT: col wl*64 + s = x[node at (win c*WPC+wl, slot s)]
        per_core.append({
            "msgs": msgs_pt,
            "dstloc": dst_pt,
            "wts": w_pt,
            "lsidx": ls_idx,
            "lsw": w_pt.astype(BF16),
            "qidx": _wrap_idx(q2),
            "dstloc2": to_pt(s_dstloc2, np.float32, T2),
            "wts2": to_pt(s_w2, np.float32, T2),
        })

    colmap = np.full((NCORES, WROWS), -1, dtype=np.int64)
    colmap[score, srow] = np.arange(N)
    for c in range(NCORES):
        cm = colmap[c]
        xT = np.zeros((D, WROWS), dtype=BF16)
        used = cm >= 0
        xT[:, used] = x[cm[used]].T.astype(BF16)
        per_core[c]["xT"] = xT
    return per_core, K1, K2, win, slot


def _l2_chunks(K2):
    """Split the 824 global window pairs into gather chunks of whole
    PSUM groups (L2_GROUP_PAIRS pairs) with <= L2_TILE_CAP tiles.

    Two phases: first every core-range's local pairs [0, RS_SPLIT) (feeds
    the early ReduceScatter), then the rest."""
    pair_tiles = K2.reshape(NWIN // 2, 2).sum(axis=1)
    ranges = [(PAIRS * c, PAIRS * c + RS_SPLIT) for c in range(NCORES)]
    ranges += [(PAIRS * c + RS_SPLIT, PAIRS * (c + 1)) for c in range(NCORES)]
    chunks = []  # (pair0, npair, ntiles)
    for r0, r1 in ranges:
        p0 = r0
        while p0 < r1:
            p1 = p0
            tiles = 0
            while p1 < r1 and p1 - p0 + L2_GROUP_PAIRS <= 56:
                g1 = min(p1 + L2_GROUP_PAIRS, r1)
                add = int(pair_tiles[p1:g1].sum())
                if tiles + add > L2_TILE_CAP and tiles > 0:
                    break
                tiles += add
                p1 = g1
            chunks.append((p0, p1 - p0, tiles))
            p0 = p1
    return chunks


def _build_program(K1, K2, chunks):
    import concourse.bass as bass
    import concourse.tile as tile
    from concourse import bacc, mybir
    from concourse import library_config

    T1 = WPC * K1
    tile0 = np.zeros(NWIN + 1, dtype=np.int64)
    tile0[1:] = np.cumsum(K2)
    T2 = int(tile0[-1])

    nc = bacc.Bacc("TRN2", target_bir_lowering=False, debug=False,
                   num_devices=NCORES)
    dt = mybir.dt

    msgs_d = nc.dram_tensor("msgs", [P, T1 * D], dt.bfloat16, kind="ExternalInput")
    dstloc_d = nc.dram_tensor("dstloc", [P, T1], dt.float32, kind="ExternalInput")
    wts_d = nc.dram_tensor("wts", [P, T1], dt.float32, kind="ExternalInput")
    lsidx_d = nc.dram_tensor("lsidx", [P, T1], dt.int16, kind="ExternalInput")
    lsw_d = nc.dram_tensor("lsw", [P, T1], dt.bfloat16, kind="ExternalInput")
    qidx_d = nc.dram_tensor("qidx", [P, T2 * 8], dt.int16, kind="ExternalInput")
    dstloc2_d = nc.dram_tensor("dstloc2", [P, T2], dt.float32, kind="ExternalInput")
    wts2_d = nc.dram_tensor("wts2", [P, T2], dt.float32, kind="ExternalInput")
    xT_d = nc.dram_tensor("xT", [D, WROWS], dt.bfloat16, kind="ExternalInput")
    iota_d = nc.dram_tensor("iota", [P, WIN], dt.bfloat16, kind="ExternalInput")
    ident_d = nc.dram_tensor("ident", [P, P], dt.bfloat16, kind="ExternalInput")
    w1l_d = nc.dram_tensor("w1lT", [D, D], dt.bfloat16, kind="ExternalInput")
    w1r_d = nc.dram_tensor("w1rT", [D, D], dt.bfloat16, kind="ExternalInput")
    w2l_d = nc.dram_tensor("w2lT", [D, D], dt.bfloat16, kind="ExternalInput")
    w2r_d = nc.dram_tensor("w2rT", [D, D], dt.bfloat16, kind="ExternalInput")
    b1_d = nc.dram_tensor("b1c", [D, 1], dt.float32, kind="ExternalInput")
    b2_d = nc.dram_tensor("b2c", [D, 1], dt.float32, kind="ExternalInput")
    out_d = nc.dram_tensor("out", [D, WROWS], dt.float32, kind="ExternalOutput")

    # L1 chunks of CHUNK1_W windows
    l1_chunks = []
    w0 = 0
    while w0 < WPC:
        cw = min(CHUNK1_W, WPC - w0)
        l1_chunks.append((w0, cw))
        w0 += cw

    with tile.TileContext(nc) as tc:
        with (
            tc.tile_pool(name="const", bufs=1) as cpool,
            tc.tile_pool(name="dram", bufs=1, space="DRAM") as dpool,
        ):
            iota_sb = cpool.tile([P, WIN], dt.bfloat16, tag="iota")
            ident_sb = cpool.tile([P, P], dt.bfloat16, tag="ident")
            w1l_sb = cpool.tile([D, D], dt.bfloat16, tag="w1l")
            w1r_sb = cpool.tile([D, D], dt.bfloat16, tag="w1r")
            w2l_sb = cpool.tile([D, D], dt.bfloat16, tag="w2l")
            w2r_sb = cpool.tile([D, D], dt.bfloat16, tag="w2r")
            b1_sb = cpool.tile([D, 1], dt.float32, tag="b1")
            b2_sb = cpool.tile([D, 1], dt.float32, tag="b2")
            h1T_sb = cpool.tile([D, WROWS], dt.bfloat16, tag="h1T")
            h1rows_sb = cpool.tile([P, PAIRS * D], dt.bfloat16, tag="h1rows")
            qidx_sb = cpool.tile([P, T2 * 8], dt.int16, tag="qidx")
            dstloc2_sb = cpool.tile([P, T2], dt.float32, tag="dstloc2")
            wts2_sb = cpool.tile([P, T2], dt.float32, tag="wts2")

            table_dram = dpool.tile([WROWS, P], dt.bfloat16, tag="table")
            # two partial tables, one per ReduceScatter chunk; rows are
            # core-major so the flat 8-way RS split lands on core boundaries
            NPA, NPB = RS_SPLIT, PAIRS - RS_SPLIT
            partialA = dpool.tile([NCORES * NPA * P, D], dt.bfloat16, tag="pA")
            partialB = dpool.tile([NCORES * NPB * P, D], dt.bfloat16, tag="pB")
            agg2A = dpool.tile([NPA * P, D], dt.bfloat16, tag="agg2A")
            agg2B = dpool.tile([NPB * P, D], dt.bfloat16, tag="agg2B")

            # ---------------- layer 1 ----------------
            with (
                tc.tile_pool(name="l1c", bufs=1) as l1c,
                tc.tile_pool(name="ch", bufs=2) as chpool,
                tc.tile_pool(name="m1", bufs=16) as mpool,
                tc.tile_pool(name="sp1", bufs=3) as spool,
                tc.tile_pool(name="psA", bufs=2, space="PSUM") as psA,
                tc.tile_pool(name="psB", bufs=2, space="PSUM") as psB,
            ):
                dstloc_sb = l1c.tile([P, T1], dt.float32, tag="dstloc")
                wts_sb = l1c.tile([P, T1], dt.float32, tag="wts")
                lsidx_sb = l1c.tile([P, T1], dt.int16, tag="lsidx")
                lsw_sb = l1c.tile([P, T1], dt.bfloat16, tag="lsw")
                xT_sb = l1c.tile([D, WROWS], dt.bfloat16, tag="xT")

                for t_sb, t_d in [(iota_sb, iota_d), (dstloc_sb, dstloc_d),
                                  (wts_sb, wts_d)]:
                    nc.sync.dma_start(out=t_sb[:], in_=t_d.ap())
                for t_sb, t_d in [(lsidx_sb, lsidx_d), (lsw_sb, lsw_d)]:
                    nc.scalar.dma_start(out=t_sb[:], in_=t_d.ap())
                # L2 consts load on idle engines during L1 (CoreSim charges
                # DMA transfers serially to the issuing engine)
                deferred_sp = [(ident_sb, ident_d), (w1l_sb, w1l_d),
                               (w1r_sb, w1r_d), (b1_sb, b1_d),
                               (w2l_sb, w2l_d), (w2r_sb, w2r_d), (b2_sb, b2_d)]
                deferred_pool = [(qidx_sb, qidx_d), (dstloc2_sb, dstloc2_d),
                                 (wts2_sb, wts2_d)]
                deferred_act = [(xT_sb, xT_d)]

                for w0, cw in l1_chunks:
                    ch = chpool.tile([P, CHUNK1_W * K1 * D], dt.bfloat16, tag="ch")
                    nc.sync.dma_start(
                        out=ch[:, :cw * K1 * D],
                        in_=msgs_d.ap()[:, w0 * K1 * D:(w0 + cw) * K1 * D])
                    if w0 == 0:
                        for t_sb, t_d in deferred_sp:
                            nc.sync.dma_start(out=t_sb[:], in_=t_d.ap())
                        for t_sb, t_d in deferred_pool:
                            nc.gpsimd.dma_start(out=t_sb[:], in_=t_d.ap())
                        for t_sb, t_d in deferred_act:
                            nc.scalar.dma_start(out=t_sb[:], in_=t_d.ap())
                    s0 = 0
                    while s0 < cw:
                        sw = min(SUPER1_W, cw - s0)
                        agg_ps = psA.tile([D, SUPER1_W * WIN], dt.float32, tag="agg")
                        for s in range(sw):
                            wi = w0 + s0 + s
                            if wi % 2 == 0:
                                # whole-window M build on the idle Pool engine
                                mwin = mpool.tile([P, K1 * WIN], dt.bfloat16,
                                                  tag="Mw")
                                nc.gpsimd.local_scatter(
                                    out_ap=mwin[:],
                                    data_ap=lsw_sb[:, wi * K1:(wi + 1) * K1],
                                    idxs_ap=lsidx_sb[:, wi * K1:(wi + 1) * K1],
                                    channels=P, num_elems=K1 * WIN,
                                    num_idxs=K1)
                            for k in range(K1):
                                t = wi * K1 + k
                                if wi % 2 == 0:
                                    mt = mwin[:, k * WIN:(k + 1) * WIN]
                                else:
                                    m1t = mpool.tile([P, WIN], dt.bfloat16,
                                                     tag="M")
                                    nc.vector.tensor_scalar(
                                        out=m1t[:], in0=iota_sb[:],
                                        scalar1=dstloc_sb[:, t:t + 1],
                                        scalar2=wts_sb[:, t:t + 1],
                                        op0=mybir.AluOpType.is_equal,
                                        op1=mybir.AluOpType.mult)
                                    mt = m1t[:]
                                woff = s0 + s
                                nc.tensor.matmul(
                                    out=agg_ps[:, s * WIN:(s + 1) * WIN],
                                    lhsT=ch[:, (woff * K1 + k) * D:
                                            (woff * K1 + k + 1) * D],
                                    rhs=mt, start=(k == 0), stop=(k == K1 - 1))
                        agg_sb = spool.tile([D, SUPER1_W * WIN], dt.bfloat16,
                                            tag="aggsb")
                        nc.scalar.copy(out=agg_sb[:, :sw * WIN],
                                       in_=agg_ps[:, :sw * WIN])
                        h_ps = psB.tile([D, SUPER1_W * WIN], dt.float32, tag="hps")
                        wabs = w0 + s0
                        nc.tensor.matmul(out=h_ps[:, :sw * WIN], lhsT=w1l_sb[:],
                                         rhs=agg_sb[:, :sw * WIN],
                                         start=True, stop=False)
                        nc.tensor.matmul(out=h_ps[:, :sw * WIN], lhsT=w1r_sb[:],
                                         rhs=xT_sb[:, wabs * WIN:(wabs + sw) * WIN],
                                         start=False, stop=True)
                        nc.scalar.activation(
                            out=h1T_sb[:, wabs * WIN:(wabs + sw) * WIN],
                            in_=h_ps[:, :sw * WIN],
                            func=mybir.ActivationFunctionType.Relu, bias=b1_sb[:])
                        s0 += sw

                # rows for the table: pair j -> local rows j*128 + p
                for j in range(PAIRS):
                    nc.sync.dma_start_transpose(
                        out=h1rows_sb[:, j * D:(j + 1) * D],
                        in_=h1T_sb[:, j * P:(j + 1) * P])
                nc.sync.dma_start(
                    out=table_dram[:].rearrange("(j p) e -> p j e", p=P)[:, :, 0:D],
                    in_=h1rows_sb[:].rearrange("p (j f) -> p j f", f=D))

            # ---------------- layer 2 (push + ReduceScatter) ----------------
            nc.gpsimd.load_library(library_config.mlp)
            stg_engines = [nc.sync, nc.scalar]
            with (
                tc.tile_pool(name="gq", bufs=2) as gqpool,
                tc.tile_pool(name="m2", bufs=16) as m2pool,
                tc.tile_pool(name="stg", bufs=2) as stgpool,
                tc.tile_pool(name="psP", bufs=3, space="PSUM") as psP,
            ):
                for ci, (p0, npair, ntiles) in enumerate(chunks):
                    t0 = int(tile0[2 * p0])
                    gq = gqpool.tile([P, L2_TILE_CAP * P], dt.bfloat16, tag="gq")
                    nc.gpsimd.dma_gather(
                        gq[:, :ntiles * P].rearrange("p (c e) -> p c e", e=P),
                        table_dram[:], qidx_sb[:, t0 * 8:(t0 + ntiles) * 8],
                        ntiles * P, ntiles * P, P, single_packet=False)
                    stg = stgpool.tile([P, 56 * D], dt.bfloat16, tag="stg")
                    g0 = 0
                    while g0 < npair:
                        gp = min(L2_GROUP_PAIRS, npair - g0)
                        pps = psP.tile([P, L2_GROUP_PAIRS * D], dt.float32,
                                       tag="pps")
                        for pr in range(gp):
                            pair = p0 + g0 + pr
                            for h in range(2):
                                g = 2 * pair + h
                                kk = int(K2[g])
                                tg = int(tile0[g])
                                for k in range(kk):
                                    t = tg + k
                                    mt = m2pool.tile([P, WIN], dt.bfloat16,
                                                     tag="M2")
                                    nc.vector.tensor_scalar(
                                        out=mt[:], in0=iota_sb[:],
                                        scalar1=dstloc2_sb[:, t:t + 1],
                                        scalar2=wts2_sb[:, t:t + 1],
                                        op0=mybir.AluOpType.is_equal,
                                        op1=mybir.AluOpType.mult)
                                    nc.tensor.matmul(
                                        out=pps[h * WIN:(h + 1) * WIN,
                                                pr * D:(pr + 1) * D],
                                        lhsT=mt[:],
                                        rhs=gq[:, (t - t0) * P:(t - t0) * P + D],
                                        start=(k == 0), stop=(k == kk - 1))
                        nc.scalar.copy(out=stg[:, g0 * D:(g0 + gp) * D],
                                       in_=pps[:, :gp * D])
                        g0 += gp
                    c, jl = p0 // PAIRS, p0 % PAIRS
                    if jl < RS_SPLIT:
                        tgt, row0 = partialA, c * NPA + jl
                    else:
                        tgt, row0 = partialB, c * NPB + (jl - RS_SPLIT)
                    stg_engines[ci % 2].dma_start(
                        out=tgt[:].rearrange("(j p) f -> p j f", p=P)
                            [:, row0:row0 + npair, :],
                        in_=stg[:, :npair * D].rearrange("p (j f) -> p j f", f=D))

            # -------- final: W2r*h1 during the collectives, then W2l --------
            SPLIT = RS_SPLIT * P
            with (
                tc.tile_pool(name="fin", bufs=1) as fin,
                tc.tile_pool(name="psT", bufs=2, space="PSUM") as psT,
                tc.tile_pool(name="psC", bufs=2, space="PSUM") as psC,
            ):
                a2rows = fin.tile([P, PAIRS * D], dt.bfloat16, tag="a2rows")
                a2T = fin.tile([D, WROWS], dt.bfloat16, tag="a2T")
                h2r = fin.tile([D, WROWS], dt.bfloat16, tag="h2r")
                ot = fin.tile([D, WROWS], dt.float32, tag="ot")

                # h2r = W2r @ h1 + b2 — independent of the collectives
                for s0 in range(0, WROWS, 512):
                    sw = min(512, WROWS - s0)
                    h_ps = psC.tile([D, 512], dt.float32, tag="h2rps")
                    nc.tensor.matmul(out=h_ps[:, :sw], lhsT=w2r_sb[:],
                                     rhs=h1T_sb[:, s0:s0 + sw],
                                     start=True, stop=True)
                    nc.scalar.add(out=h2r[:, s0:s0 + sw], in_=h_ps[:, :sw],
                                  add=b2_sb[:])

                nc.gpsimd.collective_compute(
                    "ReduceScatter", mybir.AluOpType.add,
                    replica_groups=[list(range(NCORES))],
                    ins=[partialA[:]], outs=[agg2A[:]])
                nc.gpsimd.collective_compute(
                    "ReduceScatter", mybir.AluOpType.add,
                    replica_groups=[list(range(NCORES))],
                    ins=[partialB[:]], outs=[agg2B[:]])

                halves = [(0, RS_SPLIT, agg2A), (RS_SPLIT, PAIRS, agg2B)]
                for j0, j1, a2d in halves:
                    nc.sync.dma_start(
                        out=a2rows[:, j0 * D:j1 * D].rearrange(
                            "p (j f) -> p j f", f=D),
                        in_=a2d[:].rearrange("(j p) f -> p j f", p=P))
                    for jg in range(j0, j1, 4):
                        je = min(jg + 4, j1)
                        tr = psT.tile([D, 4 * P], dt.bfloat16, tag="tr")
                        for j in range(jg, je):
                            nc.tensor.transpose(
                                out=tr[:, (j - jg) * P:(j - jg + 1) * P],
                                in_=a2rows[:, j * D:(j + 1) * D],
                                identity=ident_sb[:])
                        nc.scalar.copy(out=a2T[:, jg * P:je * P],
                                       in_=tr[:, :(je - jg) * P])
                    for s0 in range(j0 * P, j1 * P, 512):
                        sw = min(512, j1 * P - s0)
                        h_ps = psC.tile([D, 512], dt.float32, tag="h2ps")
                        nc.tensor.matmul(out=h_ps[:, :sw], lhsT=w2l_sb[:],
                                         rhs=a2T[:, s0:s0 + sw],
                                         start=True, stop=True)
                        nc.vector.scalar_tensor_tensor(
                            out=ot[:, s0:s0 + sw], in0=h_ps[:, :sw],
                            scalar=1.0, in1=h2r[:, s0:s0 + sw],
                            op0=mybir.AluOpType.mult,
                            op1=mybir.AluOpType.add)
                    nc.sync.dma_start(out=out_d.ap()[:, j0 * P:j1 * P],
                                      in_=ot[:, j0 * P:j1 * P])

    nc.compile()
    return nc


def prepare(x, edge_index, W1l, W1r, b1, W2l, W2r, b2):
    per_core, K1, K2, win, slot = _host_prep(x, edge_index)
    iota = np.tile(np.arange(WIN, dtype=np.float32), (P, 1)).astype(BF16)
    ident = np.eye(P, dtype=np.float32).astype(BF16)
    common = {
        "iota": iota, "ident": ident,
        "w1lT": np.asarray(W1l, np.float32).T.astype(BF16).copy(),
        "w1rT": np.asarray(W1r, np.float32).T.astype(BF16).copy(),
        "w2lT": np.asarray(W2l, np.float32).T.astype(BF16).copy(),
        "w2rT": np.asarray(W2r, np.float32).T.astype(BF16).copy(),
        "b1c": np.asarray(b1, np.float32).reshape(D, 1).copy(),
        "b2c": np.asarray(b2, np.float32).reshape(D, 1).copy(),
    }
    in_maps = [{**common, **pc} for pc in per_core]
    chunks = _l2_chunks(K2)
    nc = _build_program(K1, K2, chunks)
    return nc, in_maps, win, slot


def kernel(x, edge_index, W1l, W1r, b1, W2l, W2r, b2):
    from concourse import bass_utils

    nc, in_maps, win, slot = prepare(x, edge_index, W1l, W1r, b1,
                                     W2l, W2r, b2)
    res = bass_utils.run_bass_kernel_spmd(nc, in_maps, list(range(NCORES)))

    out = np.empty((N, D), dtype=np.float32)
    cols = (win % WPC) * WIN + slot
    cores = win // WPC
    for c in range(NCORES):
        m = cores == c
        out[m] = res.results[c]["out"][:, cols[m]].T
    return out
